# revision 29
# baseline (speedup 1.0000x reference)
"""Trainium2 Bass kernel for nn_AutoCorrelation (multi-head attention with a
distance decay bias), SPMD across 8 NeuronCores.

Sharding: core = (batch b, head-group hg) with b in 0..3, hg in 0..1.
Each core computes, for its batch and its 8 heads: QKV projections
(column-sharded weights), distance-banded attention (the -0.1*|i-j| bias makes
weights beyond |i-j|=64 numerically zero at the 2e-2 gate), and a row-sharded
output projection. The host sums the two half partial outputs per batch and
adds the effective output bias.

Math notes:
 - bk drops out entirely (softmax row-shift invariance: K-side bias only adds
   per-query constants to scores).
 - bv passes through attention (softmax rows sum to 1) and is folded into the
   host-side output bias: bo_eff = bo + Wo @ bv.
 - scores are built transposed St[k, q] so the P@V matmul needs no transposes.
 - the -0.1|i-j| bias is ADDED into the score PSUM pre-exp via an identity
   matmul against a precomputed Toeplitz tile (pre-scaled by 8 to compose with
   the exp's 0.125 scale), so exp() directly yields the biased weights.
 - two k-chunks (span 256 each) share one 512-col PSUM bank: one exp / pair.
 - V carries 64 ones-columns (cols 64:128), so the PV matmul replicates the
   softmax denominators across PSUM partitions 64:128; normalization is then a
   64-lane reciprocal + one multiply, no partition broadcast.
"""

import math
from contextlib import ExitStack

import numpy as np
import ml_dtypes

BF16 = ml_dtypes.bfloat16

N_CORES = 8


class Cfg:
    def __init__(self, L=2048, C=1024, NHL=8, DK=64, W=64):
        self.L, self.C, self.NHL, self.DK, self.W = L, C, NHL, DK, W
        self.DL = NHL * DK               # local head dims
        self.SPAN = 128 + 2 * W          # k-chunk q-span
        self.KC = L // 128               # k chunks
        self.NPAIR = self.KC // 2        # k-chunk pairs (share a psum bank)
        self.NQT = L // 512              # q tiles (512)
        self.CC = C // 128               # contraction chunks
        self.LT = L // 512               # l tiles
        self.HP = NHL // 2               # head pairs
        self.VW = NHL * 128              # per-kc V width (64 data + 64 ones per head)
        assert self.SPAN % 64 == 0 and 2 * self.SPAN <= 512

    def qs_of(self, kc):
        return min(max(128 * kc - self.W, 0), self.L - self.SPAN)


FULL = Cfg(W=64)


def build_pv_plan(cfg):
    """PV matmul descriptors on 64-col q blocks. Runs split only at q-tile
    boundaries; PSUM start=True bank-zeroing makes first-touch splits
    unnecessary (only the very first MM into a po bank starts)."""
    BQ = 64
    BPT = 512 // BQ  # blocks per q tile
    pv_mms = []      # (kc, qt_i, qoff, ncols, eoff)
    qt_order = {qt: [] for qt in range(cfg.NQT)}
    per_kc = {kc: [] for kc in range(cfg.KC)}
    for kc in range(cfg.KC):
        qs = cfg.qs_of(kc)
        b0 = qs // BQ
        run = [b0]
        for b in list(range(b0 + 1, b0 + cfg.SPAN // BQ)) + [None]:
            if b is not None and b // BPT == run[0] // BPT:
                run.append(b)
            else:
                qt_i = run[0] // BPT
                mm_id = len(pv_mms)
                pv_mms.append((kc, qt_i, (run[0] % BPT) * BQ, len(run) * BQ,
                               (run[0] - b0) * BQ))
                qt_order[qt_i].append(mm_id)
                per_kc[kc].append(mm_id)
                run = [b] if b is not None else []
    qt_first = {qt: ids[0] for qt, ids in qt_order.items()}
    qt_last = {qt: ids[-1] for qt, ids in qt_order.items()}
    qt_done_at = {qt: pv_mms[ids[-1]][0] for qt, ids in qt_order.items()}
    return pv_mms, per_kc, qt_first, qt_last, qt_done_at


def build_program(cfg=FULL, debug=False, debug_dump=False):
    import concourse.bass as bass
    import concourse.tile as tile
    from concourse import bacc, mybir

    f32 = mybir.dt.float32
    bf16 = mybir.dt.bfloat16
    AF = mybir.ActivationFunctionType

    L, C, NHL, DL = cfg.L, cfg.C, cfg.NHL, cfg.DL
    SPAN, KC, NQT, CC, LT, HP, VW = (cfg.SPAN, cfg.KC, cfg.NQT, cfg.CC,
                                     cfg.LT, cfg.HP, cfg.VW)
    NPAIR = cfg.NPAIR

    nc = bacc.Bacc("TRN2", target_bir_lowering=False, debug=debug,
                   num_devices=N_CORES)

    xq = nc.dram_tensor("xq", [C, L], bf16, kind="ExternalInput").ap()
    xk = nc.dram_tensor("xk", [C, L], bf16, kind="ExternalInput").ap()
    xv = nc.dram_tensor("xv", [C, L], bf16, kind="ExternalInput").ap()
    wq = nc.dram_tensor("wq", [C, DL], bf16, kind="ExternalInput").ap()
    wk = nc.dram_tensor("wk", [C, DL], bf16, kind="ExternalInput").ap()
    wv = nc.dram_tensor("wv", [C, DL], bf16, kind="ExternalInput").ap()
    wo = nc.dram_tensor("wo", [DL, C], bf16, kind="ExternalInput").ap()
    bqd = nc.dram_tensor("bq", [DL, 1], f32, kind="ExternalInput").ap()
    tbd = nc.dram_tensor("tb2", [128, 3 * 512], bf16, kind="ExternalInput").ap()
    idd = nc.dram_tensor("ident", [128, 128], bf16, kind="ExternalInput").ap()
    out = nc.dram_tensor("out", [L, C], f32, kind="ExternalOutput").ap()
    if debug_dump:
        dbg_vb = nc.dram_tensor("dbg_vb", [128, KC * VW], bf16,
                                kind="ExternalOutput").ap()
        dbg_qt = nc.dram_tensor("dbg_qt", [128, L], bf16,
                                kind="ExternalOutput").ap()
        dbg_kt = nc.dram_tensor("dbg_kt", [128, L], bf16,
                                kind="ExternalOutput").ap()
        dbg_etb = nc.dram_tensor("dbg_etb", [128, NPAIR * 512], bf16,
                                 kind="ExternalOutput").ap()
        dbg_ots = nc.dram_tensor("dbg_ots", [128, L], bf16,
                                 kind="ExternalOutput").ap()
        dbg_po = nc.dram_tensor("dbg_po", [128, 4 * 512], f32,
                                kind="ExternalOutput").ap()

    pv_mms, per_kc, qt_first, qt_last, qt_done_at = build_pv_plan(cfg)

    with tile.TileContext(nc) as tc, ExitStack() as ctx:
        const = ctx.enter_context(tc.tile_pool(name="const", bufs=1))
        big = ctx.enter_context(tc.tile_pool(name="big", bufs=1))
        xs = ctx.enter_context(tc.tile_pool(name="xs", bufs=3))
        ets = ctx.enter_context(tc.tile_pool(name="ets", bufs=4))
        rfp = ctx.enter_context(tc.tile_pool(name="rfp", bufs=2))
        ostage = ctx.enter_context(tc.tile_pool(name="ostage", bufs=3))
        psum = ctx.enter_context(tc.tile_pool(name="psum", bufs=1, space="PSUM"))

        # ---- resident constants / activations ----
        wq_sb = const.tile([128, CC * DL], bf16)
        wk_sb = const.tile([128, CC * DL], bf16)
        wv_sb = const.tile([128, CC * DL], bf16)
        wo_sb = const.tile([128, HP * C], bf16)
        tb_sb = const.tile([128, 3 * 512], bf16)
        id_sb = const.tile([128, 128], bf16)
        bq_sb = const.tile([128, HP], f32)

        qt_sb = [big.tile([128, L], bf16, name=f"qt{hp}") for hp in range(HP)]
        kt_sb = [big.tile([128, L], bf16, name=f"kt{hp}") for hp in range(HP)]
        vb_sb = big.tile([128, KC * VW], bf16)
        ots_sb = [big.tile([128, L], bf16, name=f"ots{hp}") for hp in range(HP)]

        def dram_chunked(src, ncol):
            # [C, ncol] dram view -> [128, CC_sub, ncol] AP (partition-major)
            return src.rearrange("(c p) j -> p c j", p=128)

        x_tiles = {}

        def stage_x(which, xdram, lt, nsplit=1):
            t = xs.tile([128, CC * 512], bf16, tag="xs", name=f"x_{which}{lt}")
            dst = t.rearrange("p (c j) -> p c j", j=512)
            src = dram_chunked(xdram[:, lt * 512:(lt + 1) * 512], 512)
            step = CC // nsplit
            for s in range(0, CC, step):
                nc.sync.dma_start(dst[:, s:s + step], src[:, s:s + step])
            x_tiles[(which, lt)] = t

        # startup-ordered DMAs: interleave wq/xq(lt0) per contraction chunk so
        # the first projection matmul can start after ~2 chunk transfers
        wq_v = wq_sb.rearrange("p (c d) -> p c d", d=DL)
        xq0 = xs.tile([128, CC * 512], bf16, tag="xs", name="x_q0")
        xq0_v = xq0.rearrange("p (c j) -> p c j", j=512)
        xq0_src = dram_chunked(xq[:, 0:512], 512)
        wq_src = dram_chunked(wq, DL)
        for c in range(CC):
            nc.sync.dma_start(wq_v[:, c:c + 1], wq_src[:, c:c + 1])
            nc.sync.dma_start(xq0_v[:, c:c + 1], xq0_src[:, c:c + 1])
        x_tiles[("q", 0)] = xq0
        nc.sync.dma_start(
            bq_sb[:], bqd.rearrange("(h p) one -> p (h one)", p=128))
        wk_v = wk_sb.rearrange("p (c d) -> p c d", d=DL)
        wk_src = dram_chunked(wk, DL)
        for s in range(0, CC, 2):
            nc.sync.dma_start(wk_v[:, s:s + 2], wk_src[:, s:s + 2])
        stage_x("k", xk, 0, nsplit=4)
        wv_v = wv_sb.rearrange("p (c d) -> p c d", d=DL)
        wv_src = dram_chunked(wv, DL)
        for s in range(0, CC, 4):
            nc.sync.dma_start(wv_v[:, s:s + 4], wv_src[:, s:s + 4])
        stage_x("v", xv, 0, nsplit=2)
        nc.sync.dma_start(tb_sb[:], tbd[:])
        nc.sync.dma_start(id_sb[:], idd[:])
        for lt in range(1, LT):
            for which, xdram in (("q", xq), ("k", xk), ("v", xv)):
                stage_x(which, xdram, lt)
        nc.sync.dma_start(wo_sb.rearrange("p (h m) -> p h m", m=C),
                          wo.rearrange("(h p) m -> p h m", p=128))

        # ones columns of V (denominator replication): cols 64:128 per head
        vb_v = vb_sb.rearrange("p (k h w) -> p k h w", h=NHL, w=128)
        nc.gpsimd.memset(vb_v[:, :, :, 64:128], 1.0)

        # ================= Phase A: projections =================
        for lt in range(LT):
            for which in ("q", "k", "v"):
                x_sb = x_tiles[(which, lt)]
                if which in ("q", "k"):
                    w_sb = wq_sb if which == "q" else wk_sb
                    t_sb = qt_sb if which == "q" else kt_sb
                    for hp in range(HP):
                        # Q uses the 'sc' ring so K/V never stall on Q drains
                        ps = psum.tile([128, 512], f32,
                                       tag=("sc" if which == "q" else "one"),
                                       bufs=(2 if which == "q" else 4),
                                       name=f"psp_{which}{lt}_{hp}")
                        for c in range(CC):
                            nc.tensor.matmul(
                                ps[:],
                                lhsT=w_sb[:, c * DL + hp * 128: c * DL + hp * 128 + 128],
                                rhs=x_sb[:, c * 512:(c + 1) * 512],
                                start=(c == 0), stop=(c == CC - 1))
                        dst = t_sb[hp][:, lt * 512:(lt + 1) * 512]
                        if which == "q":
                            nc.scalar.activation(dst, ps[:], AF.Identity,
                                                 bias=bq_sb[:, hp:hp + 1], scale=1.0)
                        else:
                            nc.vector.tensor_copy(dst, ps[:])
                else:
                    for sub in range(4):
                        kcg = lt * 4 + sub
                        ps = psum.tile([128, DL], f32, tag="one", bufs=4,
                                       name=f"psp_v{kcg}")
                        for c in range(CC):
                            lhsT = x_sb[:, c * 512 + sub * 128: c * 512 + sub * 128 + 128]
                            nc.tensor.matmul(
                                ps[:], lhsT=lhsT,
                                rhs=wv_sb[:, c * DL:(c + 1) * DL],
                                start=(c == 0), stop=(c == CC - 1))
                        # scatter the compact [128, 512] projection into the
                        # 128-stride [V_h | ones] layout
                        vbk = vb_sb[:, kcg * VW:(kcg + 1) * VW].rearrange(
                            "p (h w) -> p h w", w=128)
                        nc.vector.tensor_copy(
                            vbk[:, :, 0:64],
                            ps.rearrange("p (h w) -> p h w", w=64))

        # ================= Phase B: banded attention =================
        def outproj_qt(qt_i):
            for qc in range(4 * qt_i, 4 * qt_i + 4):
                st = ostage.tile([128, C], f32, tag="fo", name=f"fo{qc}")
                for mi, mo in enumerate((0, 512)):
                    pf = psum.tile([128, 512], f32, tag="one", bufs=4,
                                   name=f"pf{qc}_{mo}")
                    for hp2 in range(HP):
                        nc.tensor.matmul(
                            pf[:],
                            lhsT=ots_sb[hp2][:, qc * 128:(qc + 1) * 128],
                            rhs=wo_sb[:, hp2 * C + mo: hp2 * C + mo + 512],
                            start=(hp2 == 0), stop=(hp2 == HP - 1))
                    if mi == 0:
                        nc.scalar.copy(st[:, mo:mo + 512], pf[:])
                    else:
                        nc.vector.tensor_copy(st[:, mo:mo + 512], pf[:])
                    # per-half DMA so the final output drain overlaps the
                    # remaining copies instead of serializing after them
                    nc.sync.dma_start(
                        out[qc * 128:(qc + 1) * 128, mo:mo + 512],
                        st[:, mo:mo + 512])

        NSUP = NPAIR // 2
        scps = {}

        def issue_bias(h, sp):
            # Toeplitz bias (pre-scaled by 8) seeds a 2-bank super-tile;
            # start=True marks each bank pending-zero. Depends only on the
            # ring, so it runs ahead of the scores (across head boundaries).
            ps = psum.tile([128, 1024], f32, tag="sc", bufs=2,
                           name=f"ps_s{h}_{sp}")
            for j in range(2):
                pp = 2 * sp + j
                v = 0 if pp == 0 else (2 if pp == NPAIR - 1 else 1)
                nc.tensor.matmul(ps[:, j * 512:(j + 1) * 512],
                                 lhsT=id_sb[:],
                                 rhs=tb_sb[:, v * 512:(v + 1) * 512],
                                 start=True, stop=False)
            scps[(h, sp)] = ps

        issue_bias(0, 0)
        for h in range(NHL):
            hp, hi = h // 2, h % 2
            po = {}
            etbs = {}

            def issue_pv(pp, h=h, hp=hp, hi=hi, po=po, etbs=etbs):
                etb_t, ebase = etbs[pp]
                for half, kc in ((0, 2 * pp), (1, 2 * pp + 1)):
                    vsl = vb_sb[:, kc * VW + h * 128: kc * VW + h * 128 + 128]
                    for mm_id in per_kc[kc]:
                        _, qt_i, qoff, ncols, eoff = pv_mms[mm_id]
                        if qt_i not in po:
                            po[qt_i] = psum.tile([128, 512], f32, tag="one",
                                                 bufs=4, name=f"po{h}_{qt_i}")
                        nc.tensor.matmul(
                            po[qt_i][:, qoff:qoff + ncols], lhsT=vsl,
                            rhs=etb_t[:, ebase + half * SPAN + eoff:
                                      ebase + half * SPAN + eoff + ncols],
                            start=(qt_first[qt_i] == mm_id),
                            stop=(qt_last[qt_i] == mm_id))
                    for qt_i in [q for q, t in po.items() if qt_done_at[q] == kc]:
                        t = po.pop(qt_i)
                        sl = (slice(hi * 64, (hi + 1) * 64),
                              slice(qt_i * 512, (qt_i + 1) * 512))
                        # rows 64:128 of t are the softmax denominators,
                        # replicated by the ones-columns of V
                        if debug_dump and h == 0:
                            pst = ostage.tile([128, 512], f32, tag="dbgpo",
                                              bufs=2, name=f"dbgpo{qt_i}")
                            nc.vector.tensor_copy(pst[:], t[:])
                            nc.sync.dma_start(
                                dbg_po[:, qt_i * 512:(qt_i + 1) * 512], pst[:])
                        # rows 64:128 are the replicated denominators; DVE
                        # custom ops cannot read PSUM, so stage them (on the
                        # otherwise-idle gpsimd), recip, multiply
                        sden = rfp.tile([64, 512], f32, tag="sden",
                                        name=f"sden{h}_{qt_i}")
                        if hi == 0:
                            nc.scalar.copy(sden[:], t[64:128, :])
                        else:
                            nc.vector.tensor_copy(sden[:], t[64:128, :])
                        r_f = rfp.tile([64, 512], f32, tag="rf",
                                       name=f"rf{h}_{qt_i}")
                        nc.vector.reciprocal_approx_fast(r_f[:], sden[:])
                        nc.vector.tensor_mul(ots_sb[hp][sl], t[0:64, :], r_f[:])
                        if h == NHL - 1:
                            outproj_qt(qt_i)

            for pp in range(NPAIR):
                sp, j = pp // 2, pp % 2
                kcA, kcB = 2 * pp, 2 * pp + 1
                qsA, qsB = cfg.qs_of(kcA), cfg.qs_of(kcB)
                ps = scps[(h, sp)]
                o = j * 512
                nc.tensor.matmul(
                    ps[:, o:o + SPAN],
                    lhsT=kt_sb[hp][hi * 64:(hi + 1) * 64, kcA * 128:(kcA + 1) * 128],
                    rhs=qt_sb[hp][hi * 64:(hi + 1) * 64, qsA: qsA + SPAN],
                    start=False, stop=False)
                nc.tensor.matmul(
                    ps[:, o + SPAN:o + 2 * SPAN],
                    lhsT=kt_sb[hp][hi * 64:(hi + 1) * 64, kcB * 128:(kcB + 1) * 128],
                    rhs=qt_sb[hp][hi * 64:(hi + 1) * 64, qsB: qsB + SPAN],
                    start=False, stop=(j == 1))
                if j == 1:
                    scps.pop((h, sp))
                    if sp + 1 < NSUP:
                        issue_bias(h, sp + 1)
                    elif h + 1 < NHL:
                        issue_bias(h + 1, 0)
                    # one exp covers both pairs (2 PSUM banks read across)
                    etb = ets.tile([128, 1024], bf16, tag="etb", bufs=3,
                                   name=f"etb{h}_{sp}")
                    nc.scalar.activation(etb[:], ps[:], AF.Exp, scale=0.125)
                    etbs[2 * sp] = (etb, 0)
                    etbs[2 * sp + 1] = (etb, 512)
                    if debug_dump and h == 0:
                        nc.sync.dma_start(
                            dbg_etb[:, sp * 1024:(sp + 1) * 1024], etb[:])
                # PV trails scores by 3 pairs so the tensor queue never drains
                # waiting on the exp
                if pp >= 3:
                    issue_pv(pp - 3)
            for pp in range(NPAIR - 3, NPAIR):
                issue_pv(pp)

        if debug_dump:
            nc.sync.dma_start(dbg_vb[:], vb_sb[:])
            nc.sync.dma_start(dbg_qt[:], qt_sb[0][:])
            nc.sync.dma_start(dbg_kt[:], kt_sb[0][:])
            nc.sync.dma_start(dbg_ots[:], ots_sb[0][:])

    nc.compile()
    return nc


def host_inputs(inputs, cfg=FULL):
    """Build the 8 per-core input maps + the host-side combine constant."""
    L, C, DL, NHL = cfg.L, cfg.C, cfg.DL, cfg.NHL
    q = np.asarray(inputs["queries"], np.float32)
    k = np.asarray(inputs["keys"], np.float32)
    v = np.asarray(inputs["values"], np.float32)
    Wq = np.asarray(inputs["Wq"], np.float32)
    Wk = np.asarray(inputs["Wk"], np.float32)
    Wv = np.asarray(inputs["Wv"], np.float32)
    Wo = np.asarray(inputs["Wo"], np.float32)
    bq = np.asarray(inputs["bq"], np.float32)
    bv = np.asarray(inputs["bv"], np.float32)
    bo = np.asarray(inputs["bo"], np.float32)
    B = q.shape[0]

    bo_eff = (bo.astype(np.float64) + Wo.astype(np.float64) @ bv.astype(np.float64)
              ).astype(np.float32)

    # paired Toeplitz bias tiles, pre-scaled by 8 (exp applies scale=0.125):
    # tb2[p, v*512 + half*SPAN + c] = -0.8*|p - c - off(v, half)|
    offs = [(0, -cfg.W), (-cfg.W, -cfg.W), (-cfg.W, -2 * cfg.W)]
    p = np.arange(128, dtype=np.float64)[:, None]
    c = np.arange(cfg.SPAN, dtype=np.float64)[None, :]
    tb2 = np.zeros((128, 3 * 512), np.float64)
    for vi, (o0, o1) in enumerate(offs):
        tb2[:, vi * 512: vi * 512 + cfg.SPAN] = -0.8 * np.abs(p - c - o0)
        tb2[:, vi * 512 + cfg.SPAN: vi * 512 + 2 * cfg.SPAN] = \
            -0.8 * np.abs(p - c - o1)
    tb2 = tb2.astype(BF16)
    ident = np.eye(128, dtype=BF16)

    xT = {}
    for b in range(B):
        xT[b] = (np.ascontiguousarray(q[b].T).astype(BF16),
                 np.ascontiguousarray(k[b].T).astype(BF16),
                 np.ascontiguousarray(v[b].T).astype(BF16))

    in_maps = []
    for core in range(N_CORES):
        b, hg = core // 2, core % 2
        sl = slice(hg * DL, (hg + 1) * DL)
        in_maps.append({
            "xq": xT[b][0], "xk": xT[b][1], "xv": xT[b][2],
            "wq": np.ascontiguousarray(Wq.T[:, sl]).astype(BF16),
            "wk": np.ascontiguousarray(Wk.T[:, sl]).astype(BF16),
            "wv": np.ascontiguousarray(Wv.T[:, sl]).astype(BF16),
            "wo": np.ascontiguousarray(Wo.T[sl, :]).astype(BF16),
            "bq": np.ascontiguousarray(bq[sl][:, None]),
            "tb2": tb2, "ident": ident,
        })
    return in_maps, bo_eff


_CACHED = {}


def _wait_devices_healthy(timeout_s=420):
    import time
    import jax
    import jax.numpy as jnp
    t0 = time.time()
    last = None
    while time.time() - t0 < timeout_s:
        try:
            for d in jax.devices():
                x = jax.device_put(np.ones((8, 8), np.float32), d)
                jnp.sum(x).block_until_ready()
            return
        except Exception as e:  # wedged worker recycles within a few minutes
            last = e
            time.sleep(15)
    raise RuntimeError(f"NeuronCores unhealthy after {timeout_s}s: {last}")


def kernel(**inputs):
    from concourse.bass_utils import run_bass_kernel_spmd

    cfg = FULL
    if "nc" not in _CACHED:
        _CACHED["nc"] = build_program(cfg)
    nc = _CACHED["nc"]

    in_maps, bo_eff = host_inputs(inputs, cfg)
    _wait_devices_healthy()
    try:
        res = run_bass_kernel_spmd(nc, in_maps, core_ids=list(range(N_CORES)))
    except Exception:
        _wait_devices_healthy()
        res = run_bass_kernel_spmd(nc, in_maps, core_ids=list(range(N_CORES)))
    B = np.asarray(inputs["queries"]).shape[0]
    out = np.zeros((B, cfg.L, cfg.C), np.float32)
    for b in range(B):
        out[b] = (res.results[2 * b]["out"] + res.results[2 * b + 1]["out"]
                  + bo_eff[None, :])
    return out


# revision 30
# speedup vs baseline: 1.1835x; 1.1835x over previous
"""Trainium2 Bass kernel for nn_AutoCorrelation (multi-head attention with a
distance decay bias), SPMD across 8 NeuronCores.

Sharding: core = (batch b, head-group hg) with b in 0..3, hg in 0..1.
Each core computes, for its batch and its 8 heads: QKV projections
(column-sharded weights), distance-banded attention (the -0.1*|i-j| bias makes
weights beyond |i-j|=64 numerically zero at the 2e-2 gate), and a row-sharded
output projection. The host sums the two half partial outputs per batch and
adds the effective output bias.

Math notes:
 - bk drops out entirely (softmax row-shift invariance: K-side bias only adds
   per-query constants to scores).
 - bv passes through attention (softmax rows sum to 1) and is folded into the
   host-side output bias: bo_eff = bo + Wo @ bv.
 - scores are built transposed St[k, q] so the P@V matmul needs no transposes.
 - the -0.1|i-j| bias is ADDED into the score PSUM pre-exp via an identity
   matmul against a precomputed Toeplitz tile (pre-scaled by 8 to compose with
   the exp's 0.125 scale), so exp() directly yields the biased weights.
 - two k-chunks (span 256 each) share one 512-col PSUM bank: one exp / pair.
 - V carries 64 ones-columns (cols 64:128), so the PV matmul replicates the
   softmax denominators across PSUM partitions 64:128; normalization is then a
   64-lane reciprocal + one multiply, no partition broadcast.
"""

import math
from contextlib import ExitStack

import numpy as np
import ml_dtypes

BF16 = ml_dtypes.bfloat16

N_CORES = 8


class Cfg:
    def __init__(self, L=2048, C=1024, NHL=8, DK=64, W=64):
        self.L, self.C, self.NHL, self.DK, self.W = L, C, NHL, DK, W
        self.DL = NHL * DK               # local head dims
        self.SPAN = 128 + 2 * W          # k-chunk q-span
        self.KC = L // 128               # k chunks
        self.NPAIR = self.KC // 2        # k-chunk pairs (share a psum bank)
        self.NQT = L // 512              # q tiles (512)
        self.CC = C // 128               # contraction chunks
        self.LT = L // 512               # l tiles
        self.HP = NHL // 2               # head pairs
        self.VW = NHL * 128              # per-kc V width (64 data + 64 ones per head)
        assert self.SPAN % 64 == 0 and 2 * self.SPAN <= 512

    def qs_of(self, kc):
        return min(max(128 * kc - self.W, 0), self.L - self.SPAN)


FULL = Cfg(W=64)


def build_pv_plan(cfg):
    """PV matmul descriptors on 64-col q blocks. Runs split only at q-tile
    boundaries; PSUM start=True bank-zeroing makes first-touch splits
    unnecessary (only the very first MM into a po bank starts)."""
    BQ = 64
    BPT = 512 // BQ  # blocks per q tile
    pv_mms = []      # (kc, qt_i, qoff, ncols, eoff)
    qt_order = {qt: [] for qt in range(cfg.NQT)}
    per_kc = {kc: [] for kc in range(cfg.KC)}
    for kc in range(cfg.KC):
        qs = cfg.qs_of(kc)
        b0 = qs // BQ
        run = [b0]
        for b in list(range(b0 + 1, b0 + cfg.SPAN // BQ)) + [None]:
            if b is not None and b // BPT == run[0] // BPT:
                run.append(b)
            else:
                qt_i = run[0] // BPT
                mm_id = len(pv_mms)
                pv_mms.append((kc, qt_i, (run[0] % BPT) * BQ, len(run) * BQ,
                               (run[0] - b0) * BQ))
                qt_order[qt_i].append(mm_id)
                per_kc[kc].append(mm_id)
                run = [b] if b is not None else []
    qt_first = {qt: ids[0] for qt, ids in qt_order.items()}
    qt_last = {qt: ids[-1] for qt, ids in qt_order.items()}
    qt_done_at = {qt: pv_mms[ids[-1]][0] for qt, ids in qt_order.items()}
    return pv_mms, per_kc, qt_first, qt_last, qt_done_at


def build_program(cfg=FULL, debug=False, debug_dump=False):
    import concourse.bass as bass
    import concourse.tile as tile
    from concourse import bacc, mybir

    f32 = mybir.dt.float32
    bf16 = mybir.dt.bfloat16
    AF = mybir.ActivationFunctionType

    L, C, NHL, DL = cfg.L, cfg.C, cfg.NHL, cfg.DL
    SPAN, KC, NQT, CC, LT, HP, VW = (cfg.SPAN, cfg.KC, cfg.NQT, cfg.CC,
                                     cfg.LT, cfg.HP, cfg.VW)
    NPAIR = cfg.NPAIR

    nc = bacc.Bacc("TRN2", target_bir_lowering=False, debug=debug,
                   num_devices=N_CORES)

    xq = nc.dram_tensor("xq", [C, L], bf16, kind="ExternalInput").ap()
    xk = nc.dram_tensor("xk", [C, L], bf16, kind="ExternalInput").ap()
    xv = nc.dram_tensor("xv", [C, L], bf16, kind="ExternalInput").ap()
    wq = nc.dram_tensor("wq", [C, DL], bf16, kind="ExternalInput").ap()
    wk = nc.dram_tensor("wk", [C, DL], bf16, kind="ExternalInput").ap()
    wv = nc.dram_tensor("wv", [C, DL], bf16, kind="ExternalInput").ap()
    wo = nc.dram_tensor("wo", [DL, C], bf16, kind="ExternalInput").ap()
    bqd = nc.dram_tensor("bq", [DL, 1], f32, kind="ExternalInput").ap()
    tbd = nc.dram_tensor("tb2", [128, 3 * 512], bf16, kind="ExternalInput").ap()
    idd = nc.dram_tensor("ident", [128, 128], bf16, kind="ExternalInput").ap()
    out = nc.dram_tensor("out", [L, C], f32, kind="ExternalOutput").ap()
    if debug_dump:
        dbg_vb = nc.dram_tensor("dbg_vb", [128, KC * VW], bf16,
                                kind="ExternalOutput").ap()
        dbg_qt = nc.dram_tensor("dbg_qt", [128, L], bf16,
                                kind="ExternalOutput").ap()
        dbg_kt = nc.dram_tensor("dbg_kt", [128, L], bf16,
                                kind="ExternalOutput").ap()
        dbg_etb = nc.dram_tensor("dbg_etb", [128, NPAIR * 512], bf16,
                                 kind="ExternalOutput").ap()
        dbg_ots = nc.dram_tensor("dbg_ots", [128, L], bf16,
                                 kind="ExternalOutput").ap()
        dbg_po = nc.dram_tensor("dbg_po", [128, 4 * 512], f32,
                                kind="ExternalOutput").ap()

    pv_mms, per_kc, qt_first, qt_last, qt_done_at = build_pv_plan(cfg)

    with tile.TileContext(nc) as tc, ExitStack() as ctx:
        const = ctx.enter_context(tc.tile_pool(name="const", bufs=1))
        big = ctx.enter_context(tc.tile_pool(name="big", bufs=1))
        xs = ctx.enter_context(tc.tile_pool(name="xs", bufs=3))
        ets = ctx.enter_context(tc.tile_pool(name="ets", bufs=4))
        rfp = ctx.enter_context(tc.tile_pool(name="rfp", bufs=2))
        ostage = ctx.enter_context(tc.tile_pool(name="ostage", bufs=3))
        psum = ctx.enter_context(tc.tile_pool(name="psum", bufs=1, space="PSUM"))

        # ---- resident constants / activations ----
        wq_sb = const.tile([128, CC * DL], bf16)
        wk_sb = const.tile([128, CC * DL], bf16)
        wv_sb = const.tile([128, CC * DL], bf16)
        wo_sb = const.tile([128, HP * C], bf16)
        tb_sb = const.tile([128, 3 * 512], bf16)
        id_sb = const.tile([128, 128], bf16)
        bq_sb = const.tile([128, HP], f32)

        qt_sb = [big.tile([128, L], bf16, name=f"qt{hp}") for hp in range(HP)]
        kt_sb = [big.tile([128, L], bf16, name=f"kt{hp}") for hp in range(HP)]
        vb_sb = big.tile([128, KC * VW], bf16)
        ots_sb = [big.tile([128, L], bf16, name=f"ots{hp}") for hp in range(HP)]

        def dram_chunked(src, ncol):
            # [C, ncol] dram view -> [128, CC_sub, ncol] AP (partition-major)
            return src.rearrange("(c p) j -> p c j", p=128)

        x_tiles = {}

        def stage_x(which, xdram, lt, nsplit=1):
            t = xs.tile([128, CC * 512], bf16, tag="xs", name=f"x_{which}{lt}")
            dst = t.rearrange("p (c j) -> p c j", j=512)
            src = dram_chunked(xdram[:, lt * 512:(lt + 1) * 512], 512)
            step = CC // nsplit
            for s in range(0, CC, step):
                nc.sync.dma_start(dst[:, s:s + step], src[:, s:s + step])
            x_tiles[(which, lt)] = t

        # startup-ordered DMAs: interleave wq/xq(lt0) per contraction chunk so
        # the first projection matmul can start after ~2 chunk transfers
        wq_v = wq_sb.rearrange("p (c d) -> p c d", d=DL)
        xq0 = xs.tile([128, CC * 512], bf16, tag="xs", name="x_q0")
        xq0_v = xq0.rearrange("p (c j) -> p c j", j=512)
        xq0_src = dram_chunked(xq[:, 0:512], 512)
        wq_src = dram_chunked(wq, DL)
        for c in range(CC):
            nc.sync.dma_start(wq_v[:, c:c + 1], wq_src[:, c:c + 1])
            nc.sync.dma_start(xq0_v[:, c:c + 1], xq0_src[:, c:c + 1])
        x_tiles[("q", 0)] = xq0
        nc.sync.dma_start(
            bq_sb[:], bqd.rearrange("(h p) one -> p (h one)", p=128))
        wk_v = wk_sb.rearrange("p (c d) -> p c d", d=DL)
        wk_src = dram_chunked(wk, DL)
        for s in range(0, CC, 2):
            nc.sync.dma_start(wk_v[:, s:s + 2], wk_src[:, s:s + 2])
        stage_x("k", xk, 0, nsplit=4)
        wv_v = wv_sb.rearrange("p (c d) -> p c d", d=DL)
        wv_src = dram_chunked(wv, DL)
        for s in range(0, CC, 4):
            nc.sync.dma_start(wv_v[:, s:s + 4], wv_src[:, s:s + 4])
        stage_x("v", xv, 0, nsplit=2)
        nc.sync.dma_start(tb_sb[:], tbd[:])
        nc.sync.dma_start(id_sb[:], idd[:])
        for lt in range(1, LT):
            for which, xdram in (("q", xq), ("k", xk), ("v", xv)):
                stage_x(which, xdram, lt)
        nc.sync.dma_start(wo_sb.rearrange("p (h m) -> p h m", m=C),
                          wo.rearrange("(h p) m -> p h m", p=128))

        # ones columns of V (denominator replication): cols 64:128 per head
        vb_v = vb_sb.rearrange("p (k h w) -> p k h w", h=NHL, w=128)
        nc.gpsimd.memset(vb_v[:, :, :, 64:128], 1.0)

        # ================= Phase A: projections =================
        for lt in range(LT):
            for which in ("q", "k", "v"):
                x_sb = x_tiles[(which, lt)]
                if which in ("q", "k"):
                    w_sb = wq_sb if which == "q" else wk_sb
                    t_sb = qt_sb if which == "q" else kt_sb
                    for hp in range(HP):
                        # Q uses the 'sc' ring so K/V never stall on Q drains
                        ps = psum.tile([128, 512], f32,
                                       tag=("sc" if which == "q" else "one"),
                                       bufs=(2 if which == "q" else 4),
                                       name=f"psp_{which}{lt}_{hp}")
                        for c in range(CC):
                            nc.tensor.matmul(
                                ps[:],
                                lhsT=w_sb[:, c * DL + hp * 128: c * DL + hp * 128 + 128],
                                rhs=x_sb[:, c * 512:(c + 1) * 512],
                                start=(c == 0), stop=(c == CC - 1))
                        dst = t_sb[hp][:, lt * 512:(lt + 1) * 512]
                        if which == "q":
                            nc.scalar.activation(dst, ps[:], AF.Identity,
                                                 bias=bq_sb[:, hp:hp + 1], scale=1.0)
                        else:
                            nc.vector.tensor_copy(dst, ps[:])
                else:
                    for sub in range(4):
                        kcg = lt * 4 + sub
                        ps = psum.tile([128, DL], f32, tag="one", bufs=4,
                                       name=f"psp_v{kcg}")
                        for c in range(CC):
                            lhsT = x_sb[:, c * 512 + sub * 128: c * 512 + sub * 128 + 128]
                            nc.tensor.matmul(
                                ps[:], lhsT=lhsT,
                                rhs=wv_sb[:, c * DL:(c + 1) * DL],
                                start=(c == 0), stop=(c == CC - 1))
                        # scatter the compact [128, 512] projection into the
                        # 128-stride [V_h | ones] layout
                        vbk = vb_sb[:, kcg * VW:(kcg + 1) * VW].rearrange(
                            "p (h w) -> p h w", w=128)
                        nc.vector.tensor_copy(
                            vbk[:, :, 0:64],
                            ps.rearrange("p (h w) -> p h w", w=64))

        # ================= Phase B: banded attention =================
        def outproj_qt(qt_i):
            for qc in range(4 * qt_i, 4 * qt_i + 4):
                st = ostage.tile([128, C], f32, tag="fo", name=f"fo{qc}")
                for mi, mo in enumerate((0, 512)):
                    pf = psum.tile([128, 512], f32, tag="one", bufs=4,
                                   name=f"pf{qc}_{mo}")
                    for hp2 in range(HP):
                        nc.tensor.matmul(
                            pf[:],
                            lhsT=ots_sb[hp2][:, qc * 128:(qc + 1) * 128],
                            rhs=wo_sb[:, hp2 * C + mo: hp2 * C + mo + 512],
                            start=(hp2 == 0), stop=(hp2 == HP - 1))
                    if mi == 0:
                        nc.scalar.copy(st[:, mo:mo + 512], pf[:])
                    else:
                        nc.vector.tensor_copy(st[:, mo:mo + 512], pf[:])
                nc.sync.dma_start(out[qc * 128:(qc + 1) * 128, :], st[:])

        NSUP = NPAIR // 2
        scps = {}

        def issue_bias(h, sp):
            # Toeplitz bias (pre-scaled by 8) seeds a 2-bank super-tile;
            # start=True marks each bank pending-zero. Depends only on the
            # ring, so it runs ahead of the scores (across head boundaries).
            ps = psum.tile([128, 1024], f32, tag="sc", bufs=2,
                           name=f"ps_s{h}_{sp}")
            for j in range(2):
                pp = 2 * sp + j
                v = 0 if pp == 0 else (2 if pp == NPAIR - 1 else 1)
                nc.tensor.matmul(ps[:, j * 512:(j + 1) * 512],
                                 lhsT=id_sb[:],
                                 rhs=tb_sb[:, v * 512:(v + 1) * 512],
                                 start=True, stop=False)
            scps[(h, sp)] = ps

        issue_bias(0, 0)
        for h in range(NHL):
            hp, hi = h // 2, h % 2
            po = {}
            etbs = {}

            def issue_pv(pp, h=h, hp=hp, hi=hi, po=po, etbs=etbs):
                etb_t, ebase = etbs[pp]
                for half, kc in ((0, 2 * pp), (1, 2 * pp + 1)):
                    vsl = vb_sb[:, kc * VW + h * 128: kc * VW + h * 128 + 128]
                    for mm_id in per_kc[kc]:
                        _, qt_i, qoff, ncols, eoff = pv_mms[mm_id]
                        if qt_i not in po:
                            po[qt_i] = psum.tile([128, 512], f32, tag="one",
                                                 bufs=4, name=f"po{h}_{qt_i}")
                        nc.tensor.matmul(
                            po[qt_i][:, qoff:qoff + ncols], lhsT=vsl,
                            rhs=etb_t[:, ebase + half * SPAN + eoff:
                                      ebase + half * SPAN + eoff + ncols],
                            start=(qt_first[qt_i] == mm_id),
                            stop=(qt_last[qt_i] == mm_id))
                    for qt_i in [q for q, t in po.items() if qt_done_at[q] == kc]:
                        t = po.pop(qt_i)
                        sl = (slice(hi * 64, (hi + 1) * 64),
                              slice(qt_i * 512, (qt_i + 1) * 512))
                        # rows 64:128 of t are the softmax denominators,
                        # replicated by the ones-columns of V
                        if debug_dump and h == 0:
                            pst = ostage.tile([128, 512], f32, tag="dbgpo",
                                              bufs=2, name=f"dbgpo{qt_i}")
                            nc.vector.tensor_copy(pst[:], t[:])
                            nc.sync.dma_start(
                                dbg_po[:, qt_i * 512:(qt_i + 1) * 512], pst[:])
                        # rows 64:128 are the replicated denominators; DVE
                        # custom ops cannot read PSUM, so stage them (on the
                        # otherwise-idle gpsimd), recip, multiply
                        sden = rfp.tile([64, 512], f32, tag="sden",
                                        name=f"sden{h}_{qt_i}")
                        if hi == 0:
                            nc.scalar.copy(sden[:], t[64:128, :])
                        else:
                            nc.vector.tensor_copy(sden[:], t[64:128, :])
                        r_f = rfp.tile([64, 512], f32, tag="rf",
                                       name=f"rf{h}_{qt_i}")
                        nc.vector.reciprocal_approx_fast(r_f[:], sden[:])
                        nc.vector.tensor_mul(ots_sb[hp][sl], t[0:64, :], r_f[:])
                        if h == NHL - 1:
                            outproj_qt(qt_i)

            for pp in range(NPAIR):
                sp, j = pp // 2, pp % 2
                kcA, kcB = 2 * pp, 2 * pp + 1
                qsA, qsB = cfg.qs_of(kcA), cfg.qs_of(kcB)
                ps = scps[(h, sp)]
                o = j * 512
                nc.tensor.matmul(
                    ps[:, o:o + SPAN],
                    lhsT=kt_sb[hp][hi * 64:(hi + 1) * 64, kcA * 128:(kcA + 1) * 128],
                    rhs=qt_sb[hp][hi * 64:(hi + 1) * 64, qsA: qsA + SPAN],
                    start=False, stop=False)
                nc.tensor.matmul(
                    ps[:, o + SPAN:o + 2 * SPAN],
                    lhsT=kt_sb[hp][hi * 64:(hi + 1) * 64, kcB * 128:(kcB + 1) * 128],
                    rhs=qt_sb[hp][hi * 64:(hi + 1) * 64, qsB: qsB + SPAN],
                    start=False, stop=(j == 1))
                if j == 1:
                    scps.pop((h, sp))
                    if sp + 1 < NSUP:
                        issue_bias(h, sp + 1)
                    elif h + 1 < NHL:
                        issue_bias(h + 1, 0)
                    # one exp covers both pairs (2 PSUM banks read across)
                    etb = ets.tile([128, 1024], bf16, tag="etb", bufs=3,
                                   name=f"etb{h}_{sp}")
                    nc.scalar.activation(etb[:], ps[:], AF.Exp, scale=0.125)
                    etbs[2 * sp] = (etb, 0)
                    etbs[2 * sp + 1] = (etb, 512)
                    if debug_dump and h == 0:
                        nc.sync.dma_start(
                            dbg_etb[:, sp * 1024:(sp + 1) * 1024], etb[:])
                # PV trails scores by 3 pairs so the tensor queue never drains
                # waiting on the exp
                if pp >= 3:
                    issue_pv(pp - 3)
            for pp in range(NPAIR - 3, NPAIR):
                issue_pv(pp)

        if debug_dump:
            nc.sync.dma_start(dbg_vb[:], vb_sb[:])
            nc.sync.dma_start(dbg_qt[:], qt_sb[0][:])
            nc.sync.dma_start(dbg_kt[:], kt_sb[0][:])
            nc.sync.dma_start(dbg_ots[:], ots_sb[0][:])

    nc.compile()
    return nc


def host_inputs(inputs, cfg=FULL):
    """Build the 8 per-core input maps + the host-side combine constant."""
    L, C, DL, NHL = cfg.L, cfg.C, cfg.DL, cfg.NHL
    q = np.asarray(inputs["queries"], np.float32)
    k = np.asarray(inputs["keys"], np.float32)
    v = np.asarray(inputs["values"], np.float32)
    Wq = np.asarray(inputs["Wq"], np.float32)
    Wk = np.asarray(inputs["Wk"], np.float32)
    Wv = np.asarray(inputs["Wv"], np.float32)
    Wo = np.asarray(inputs["Wo"], np.float32)
    bq = np.asarray(inputs["bq"], np.float32)
    bv = np.asarray(inputs["bv"], np.float32)
    bo = np.asarray(inputs["bo"], np.float32)
    B = q.shape[0]

    bo_eff = (bo.astype(np.float64) + Wo.astype(np.float64) @ bv.astype(np.float64)
              ).astype(np.float32)

    # paired Toeplitz bias tiles, pre-scaled by 8 (exp applies scale=0.125):
    # tb2[p, v*512 + half*SPAN + c] = -0.8*|p - c - off(v, half)|
    offs = [(0, -cfg.W), (-cfg.W, -cfg.W), (-cfg.W, -2 * cfg.W)]
    p = np.arange(128, dtype=np.float64)[:, None]
    c = np.arange(cfg.SPAN, dtype=np.float64)[None, :]
    tb2 = np.zeros((128, 3 * 512), np.float64)
    for vi, (o0, o1) in enumerate(offs):
        tb2[:, vi * 512: vi * 512 + cfg.SPAN] = -0.8 * np.abs(p - c - o0)
        tb2[:, vi * 512 + cfg.SPAN: vi * 512 + 2 * cfg.SPAN] = \
            -0.8 * np.abs(p - c - o1)
    tb2 = tb2.astype(BF16)
    ident = np.eye(128, dtype=BF16)

    xT = {}
    for b in range(B):
        xT[b] = (np.ascontiguousarray(q[b].T).astype(BF16),
                 np.ascontiguousarray(k[b].T).astype(BF16),
                 np.ascontiguousarray(v[b].T).astype(BF16))

    in_maps = []
    for core in range(N_CORES):
        b, hg = core // 2, core % 2
        sl = slice(hg * DL, (hg + 1) * DL)
        in_maps.append({
            "xq": xT[b][0], "xk": xT[b][1], "xv": xT[b][2],
            "wq": np.ascontiguousarray(Wq.T[:, sl]).astype(BF16),
            "wk": np.ascontiguousarray(Wk.T[:, sl]).astype(BF16),
            "wv": np.ascontiguousarray(Wv.T[:, sl]).astype(BF16),
            "wo": np.ascontiguousarray(Wo.T[sl, :]).astype(BF16),
            "bq": np.ascontiguousarray(bq[sl][:, None]),
            "tb2": tb2, "ident": ident,
        })
    return in_maps, bo_eff


_CACHED = {}


def _wait_devices_healthy(timeout_s=420):
    import time
    import jax
    import jax.numpy as jnp
    t0 = time.time()
    last = None
    while time.time() - t0 < timeout_s:
        try:
            for d in jax.devices():
                x = jax.device_put(np.ones((8, 8), np.float32), d)
                jnp.sum(x).block_until_ready()
            return
        except Exception as e:  # wedged worker recycles within a few minutes
            last = e
            time.sleep(15)
    raise RuntimeError(f"NeuronCores unhealthy after {timeout_s}s: {last}")


def kernel(**inputs):
    from concourse.bass_utils import run_bass_kernel_spmd

    cfg = FULL
    if "nc" not in _CACHED:
        _CACHED["nc"] = build_program(cfg)
    nc = _CACHED["nc"]

    in_maps, bo_eff = host_inputs(inputs, cfg)
    _wait_devices_healthy()
    try:
        res = run_bass_kernel_spmd(nc, in_maps, core_ids=list(range(N_CORES)))
    except Exception:
        _wait_devices_healthy()
        res = run_bass_kernel_spmd(nc, in_maps, core_ids=list(range(N_CORES)))
    B = np.asarray(inputs["queries"]).shape[0]
    out = np.zeros((B, cfg.L, cfg.C), np.float32)
    for b in range(B):
        out[b] = (res.results[2 * b]["out"] + res.results[2 * b + 1]["out"]
                  + bo_eff[None, :])
    return out
